# revision 1
# baseline (speedup 1.0000x reference)
"""Self-contained kernel for nn_Attention_17300128268584.

Strategy (per sharding hint): data-parallel over batch B=16 across the
8 NeuronCores (B=2 per core); weights/table replicated. The full
conv->GN->GELU projections, relative-position-biased attention, and the
1x1 output projection run on-device per shard; results are gathered and
concatenated on the host.

Hardcoded problem shape: x (16,128,32,32) f32, 4 heads x 32 dim,
N = 32*32 = 1024 tokens.
"""

import numpy as np

EPS = 1e-6
DIM_HEAD = 32
N_CORES = 8


def _shard_fn(x, wq, gq, bq, wk, gk, bk, wv, gv, bv, table, rel_index, out_w, out_b):
    # Runs on ONE core with a batch shard x: (2, 128, 32, 32).
    import jax
    import jax.numpy as jnp

    B, C, ih, iw = x.shape
    H = C // DIM_HEAD
    N = ih * iw

    def conv3x3(t, w):
        return jax.lax.conv_general_dilated(
            t, w, window_strides=(1, 1), padding=((1, 1), (1, 1)),
            dimension_numbers=("NCHW", "OIHW", "NCHW"))

    def gn1(t, gamma, beta):
        m = jnp.mean(t, axis=(1, 2, 3), keepdims=True)
        v = jnp.var(t, axis=(1, 2, 3), keepdims=True)
        tn = (t - m) * jax.lax.rsqrt(v + EPS)
        return tn * gamma[None, :, None, None] + beta[None, :, None, None]

    def proj(t, w, gamma, beta):
        return jax.nn.gelu(gn1(conv3x3(t, w), gamma, beta), approximate=False)

    def to_heads(t):
        return (t.reshape(B, C, N).transpose(0, 2, 1)
                 .reshape(B, N, H, DIM_HEAD).transpose(0, 2, 1, 3))

    q = to_heads(proj(x, wq, gq, bq))
    k = to_heads(proj(x, wk, gk, bk))
    v = to_heads(proj(x, wv, gv, bv))

    dots = jnp.einsum("bhnd,bhmd->bhnm", q, k)
    bias = table[rel_index].reshape(N, N, H).transpose(2, 0, 1)[None]
    attn = jax.nn.softmax(dots + bias, axis=-1)
    out = jnp.einsum("bhnm,bhmd->bhnd", attn, v)
    out = (out.transpose(0, 2, 1, 3).reshape(B, N, C)
              .transpose(0, 2, 1).reshape(B, C, ih, iw))
    out = jax.lax.conv_general_dilated(
        out, out_w, window_strides=(1, 1), padding=((0, 0), (0, 0)),
        dimension_numbers=("NCHW", "OIHW", "NCHW")) + out_b[None, :, None, None]
    return out


def _run_device(inputs):
    import jax

    jax.config.update("jax_default_matmul_precision", "highest")
    devs = [d for d in jax.devices() if d.platform != "cpu"]
    if len(devs) < N_CORES:
        raise RuntimeError(f"need {N_CORES} accelerator cores, have {len(devs)}")
    devs = devs[:N_CORES]

    x = np.asarray(inputs["x"], np.float32)
    B = x.shape[0]
    assert B % N_CORES == 0
    xs = x.reshape(N_CORES, B // N_CORES, *x.shape[1:])

    names = ["wq", "gq", "bq", "wk", "gk", "bk", "wv", "gv", "bv",
             "table", "rel_index", "out_w", "out_b"]
    reps = [np.asarray(inputs[n]) for n in names]

    pm = jax.pmap(
        _shard_fn,
        in_axes=(0,) + (None,) * len(names),
        devices=devs,
    )
    out = pm(xs, *reps)
    out = np.asarray(out, np.float32)
    return out.reshape(B, *out.shape[2:])


def _run_host(inputs):
    import jax

    with jax.default_device(jax.devices("cpu")[0]):
        out = jax.jit(_shard_fn, backend="cpu")(
            *[np.asarray(inputs[n]) for n in
              ["x", "wq", "gq", "bq", "wk", "gk", "bk", "wv", "gv", "bv",
               "table", "rel_index", "out_w", "out_b"]])
    return np.asarray(out, np.float32)


def kernel(**inputs) -> np.ndarray:
    try:
        return _run_device(inputs)
    except Exception:
        return _run_host(inputs)
